# revision 2
# baseline (speedup 1.0000x reference)
"""GAT attention layer (B=8, N=2048, C=512) on 8 TRN2 NeuronCores.

Data-parallel over B: core b handles graph b.
Per-core math (x: [N,C], w: [C,C], a: [2C,1]):
    wa_t = w @ a_t                      (t=0,1)       [C]
    s_t  = x @ wa_t                                   [N]
    p_ji = exp(leaky_relu(s1_i + s2_j))
         = max(exp(s1_i)exp(s2_j), exp(a*s1_i)exp(a*s2_j))   <- separable!
    r_i  = sum_j p_ji
    out  = (p^T @ x) / r

The separable form removes the per-element exp (would be 64us of ACT):
only 4 exps of [N]-vectors are needed; the N*N work is one broadcast
multiply (u = E1_i*F1_j) plus one fused multiply+max on DVE.
Scores kept transposed [j, i] so p slices serve directly as matmul
stationary operands (out[i,c] = sum_j p[j,i] x[j,c]); r from rhs=ones MMs.

Engine budget per core (measured costs):
  PE   : 256 PV MMs (215ns) + 256 r-MMs (25ns)  ~62us   <- pacer
  DVE  : dots 32x613ns, u-odd 8x594, p-STT 16x~1200, recip
  ACT  : u-even 8x1990, F-exps 32x190, E-exps, normalize 16x570
  GPS  : x->bf16 casts 16x1913 + small broadcast DMAs
"""

import sys

import numpy as np

if "/opt/trn_rl_repo" not in sys.path:
    sys.path.insert(0, "/opt/trn_rl_repo")

B, N, C = 8, 2048, 512
P = 128
NJ = N // P  # 16 source-node blocks
ALPHA = 0.2  # leaky_relu slope
# PSUM: 8 banks of [128, 512] fp32. 7 output accumulators + 1 r bank.
GROUPS = [(0, 7), (7, 14), (14, 16)]

_CACHE = {}


def _build():
    from contextlib import ExitStack

    import concourse.bacc as bacc
    import concourse.bass as bass
    import concourse.tile as tile
    from concourse import mybir

    fp32 = mybir.dt.float32
    bf16 = mybir.dt.bfloat16
    AF = mybir.ActivationFunctionType
    OP = mybir.AluOpType

    nc = bacc.Bacc("TRN2", target_bir_lowering=False)
    x_d = nc.dram_tensor("x", [N, C], fp32, kind="ExternalInput")
    w_d = nc.dram_tensor("w", [C, C], fp32, kind="ExternalInput")
    a_d = nc.dram_tensor("a", [2 * C, 1], fp32, kind="ExternalInput")
    o_d = nc.dram_tensor("o", [N, C], fp32, kind="ExternalOutput")

    with ExitStack() as ctx:
        tc = ctx.enter_context(tile.TileContext(nc))
        const = ctx.enter_context(tc.tile_pool(name="const", bufs=1))
        wpool = ctx.enter_context(tc.tile_pool(name="w", bufs=4))
        xpool = ctx.enter_context(tc.tile_pool(name="xin", bufs=NJ))
        xbfp = ctx.enter_context(tc.tile_pool(name="xbf", bufs=NJ))
        ppool = ctx.enter_context(tc.tile_pool(name="p", bufs=NJ))
        upool = ctx.enter_context(tc.tile_pool(name="u", bufs=3))
        scr = ctx.enter_context(tc.tile_pool(name="scr", bufs=3))
        osb = ctx.enter_context(tc.tile_pool(name="osb", bufs=3))
        dram = ctx.enter_context(tc.tile_pool(name="dram", bufs=1, space="DRAM"))
        ps_out = ctx.enter_context(tc.tile_pool(name="ps_out", bufs=7, space="PSUM"))
        ps_r = ctx.enter_context(tc.tile_pool(name="ps_r", bufs=1, space="PSUM"))

        # --- persistent small tiles -------------------------------------
        s1col = const.tile([P, NJ], fp32)  # s1[128j+p] at [p, j]
        s2col = const.tile([P, NJ], fp32)
        F1col = const.tile([P, NJ], fp32)  # exp(s2)
        F1acol = const.tile([P, NJ], fp32)  # exp(ALPHA*s2)
        Ecols = const.tile([P, 2, NJ], bf16)  # [exp(s1), exp(ALPHA*s1)] col form
        Eb = const.tile([P, 2, N], bf16)  # E rows broadcast to 128 parts
        wa12 = const.tile([P, 8], fp32)  # wa_t[128q+p] at [p, t*4+q]
        abc = const.tile([P, 2, C], fp32)  # a rows broadcast to 128 parts
        wab = const.tile([P, 2, C], fp32)  # wa rows broadcast to 128 parts
        ones_bf = const.tile([P, 1], bf16)
        ones_f32 = const.tile([P, P], fp32)
        warm_rhs = const.tile([P, C], bf16)
        rinv = const.tile([P, NJ], fp32)
        dummy = const.tile([P, 1], fp32)
        dummy2 = const.tile([P, 1], fp32)

        scratch_wa = dram.tile([2 * C], fp32)
        scratch_E = dram.tile([2 * N], bf16)

        nc.vector.memset(ones_bf[:], 1.0)
        nc.vector.memset(ones_f32[:], 1.0)
        nc.vector.memset(warm_rhs[:], 0.0)

        # Preload ACT exp table (~2.7us) during the DMA head phase.
        nc.scalar.activation(dummy[:], ones_bf[:], AF.Exp)

        # PE warm-up: ~16 back-to-back MMs trip the HAM activity window so
        # the array is at 8/8 clock before real matmuls arrive.
        warm_ps = ps_r.tile([P, C], fp32, tag="rps", name="warm_ps")
        for _ in range(16):
            nc.tensor.matmul(
                warm_ps[0:1, :],
                lhsT=ones_bf[:],
                rhs=warm_rhs[:],
                start=True,
                stop=True,
                skip_group_check=True,
            )

        # --- a -> abc (partition-broadcast DMA straight from DRAM) ------
        a_rows = a_d[:, 0].rearrange("(t c) -> t c", t=2)  # [2, C]
        nc.gpsimd.dma_start(
            out=abc[:],
            in_=bass.AP(
                tensor=a_rows.tensor,
                offset=a_rows.offset,
                ap=[[0, P]] + list(a_rows.ap),
            ),
        )

        # --- load w (gpsimd queue: ahead of the casts), compute wa -------
        wt = []
        for q in range(4):
            t = wpool.tile([P, C], fp32, tag="w")
            nc.gpsimd.dma_start(t[:], w_d[q * P : (q + 1) * P, :])
            wt.append(t)
        for q in range(4):
            for t in range(2):
                s = scr.tile([P, C], fp32, tag="ttr")
                nc.vector.scalar_tensor_tensor(
                    out=s[:],
                    in0=wt[q][:],
                    scalar=0.0,
                    in1=abc[:, t, :],
                    op0=OP.add,
                    op1=OP.mult,
                    accum_out=wa12[:, t * 4 + q : t * 4 + q + 1],
                )
        # wa12 -> DRAM at [t*C + 128q + p], then broadcast back as rows
        nc.gpsimd.dma_start(
            out=scratch_wa[:].rearrange("(t q p) -> p t q", t=2, p=P),
            in_=wa12[:].rearrange("p (t q) -> p t q", t=2),
        )
        wa_rows = scratch_wa[:].rearrange("(t c) -> t c", t=2)
        nc.gpsimd.dma_start(
            out=wab[:],
            in_=bass.AP(
                tensor=wa_rows.tensor,
                offset=wa_rows.offset,
                ap=[[0, P]] + list(wa_rows.ap),
            ),
        )

        # --- load x; s1 row-dots on DVE (chase the DMA); casts on GPSIMD;
        #     keep-alive MMs on PE (rhs=xin so they spread with the DMAs) -
        xin, xbf = [], []
        for j in range(NJ):
            t = xpool.tile([P, C], fp32, tag="xin")
            nc.sync.dma_start(t[:], x_d[j * P : (j + 1) * P, :])
            xin.append(t)
            for _ in range(2):
                nc.tensor.matmul(
                    warm_ps[:, :],
                    lhsT=ones_f32[:],
                    rhs=t[:],
                    start=True,
                    stop=True,
                    skip_group_check=True,
                )
            s = scr.tile([P, C], fp32, tag="ttr")
            nc.vector.scalar_tensor_tensor(
                out=s[:],
                in0=t[:],
                scalar=0.0,
                in1=wab[:, 0, :],
                op0=OP.add,
                op1=OP.mult,
                accum_out=s1col[:, j : j + 1],
            )
            xb = xbfp.tile([P, C], bf16, tag="xbf")
            nc.gpsimd.tensor_copy(xb[:], t[:])
            xbf.append(xb)

        # Preload DVE reciprocal table off the critical path.
        nc.vector.reciprocal(dummy2[:], dummy[:])

        # --- E vectors: exp(s1), exp(a*s1) -> scatter -> broadcast rows --
        nc.scalar.activation(Ecols[:, 0, :], s1col[:], AF.Exp)
        nc.scalar.activation(Ecols[:, 1, :], s1col[:], AF.Exp, scale=ALPHA)
        nc.tensor.matmul(  # PE keep-alive blip, depends on Ecols
            warm_ps[0:1, :32],
            lhsT=ones_bf[:],
            rhs=Ecols[:].rearrange("p t j -> p (t j)"),
            start=True,
            stop=True,
            skip_group_check=True,
        )
        nc.sync.dma_start(
            out=scratch_E[:].rearrange("(t j p) -> p t j", t=2, p=P),
            in_=Ecols[:],
        )
        e_rows = scratch_E[:].rearrange("(t c) -> t c", t=2)
        nc.sync.dma_start(
            out=Eb[:],
            in_=bass.AP(
                tensor=e_rows.tensor,
                offset=e_rows.offset,
                ap=[[0, P]] + list(e_rows.ap),
            ),
        )
        nc.tensor.matmul(  # PE keep-alive blip, depends on Eb broadcast
            warm_ps[0:1, :],
            lhsT=ones_bf[:],
            rhs=Eb[:, 0, 0:C],
            start=True,
            stop=True,
            skip_group_check=True,
        )

        # s2 row-dots + F exps, interleaved into the stream with lookahead
        # so block j's scalars are ready when its u/p ops run.
        def emit_s2(j):
            s = scr.tile([P, C], fp32, tag="ttr", name=f"s2scr_{j}")
            nc.vector.scalar_tensor_tensor(
                out=s[:],
                in0=xin[j][:],
                scalar=0.0,
                in1=wab[:, 1, :],
                op0=OP.add,
                op1=OP.mult,
                accum_out=s2col[:, j : j + 1],
            )
            nc.scalar.activation(F1col[:, j : j + 1], s2col[:, j : j + 1], AF.Exp)
            nc.scalar.activation(
                F1acol[:, j : j + 1], s2col[:, j : j + 1], AF.Exp, scale=ALPHA
            )

        emit_s2(0)
        emit_s2(1)

        # --- scores stream: u = E1_i*F1_j ; p = max(E1a_i*F1a_j, u) ------
        pt = []
        for j in range(NJ):
            u = upool.tile([P, N], bf16, tag="u")
            if j % 2 == 0:
                nc.scalar.activation(
                    u[:], Eb[:, 0, :], AF.Copy, bias=0.0, scale=F1col[:, j : j + 1]
                )
            else:
                nc.vector.tensor_scalar_mul(u[:], Eb[:, 0, :], F1col[:, j : j + 1])
            p = ppool.tile([P, N], bf16, tag="p")
            nc.vector.scalar_tensor_tensor(
                out=p[:],
                in0=Eb[:, 1, :],
                scalar=F1acol[:, j : j + 1],
                in1=u[:],
                op0=OP.mult,
                op1=OP.max,
            )
            pt.append(p)
            if j + 2 < NJ:
                emit_s2(j + 2)

        # --- PV + r + normalize, in PSUM-sized chunk groups --------------
        for g0, g1 in GROUPS:
            nk = g1 - g0
            outps = [
                ps_out.tile([P, C], fp32, tag="ops", name=f"ops_{g0}_{ki}")
                for ki in range(nk)
            ]
            rps = ps_r.tile([P, C], fp32, tag="rps")
            for j in range(NJ):
                first, last = j == 0, j == NJ - 1
                for ki, k in enumerate(range(g0, g1)):
                    lhs = pt[j][:, k * P : (k + 1) * P]
                    nc.tensor.matmul(
                        outps[ki][:], lhsT=lhs, rhs=xbf[j][:], start=first, stop=last
                    )
                    # start=True clears the WHOLE bank's has_written bits, so
                    # only the very first matmul into this bank may set it;
                    # later first-touches per element overwrite (bit clear)
                    # and the rest accumulate.
                    nc.tensor.matmul(
                        rps[:, ki : ki + 1],
                        lhsT=lhs,
                        rhs=ones_bf[:],
                        start=first and ki == 0,
                        stop=last,
                        skip_group_check=True,
                    )
            nc.vector.reciprocal(rinv[:, g0:g1], rps[:, :nk])
            for ki, k in enumerate(range(g0, g1)):
                ob = osb.tile([P, C], fp32, tag="ob")
                nc.scalar.activation(
                    ob[:], outps[ki][:], AF.Copy, bias=0.0, scale=rinv[:, k : k + 1]
                )
                nc.sync.dma_start(o_d[k * P : (k + 1) * P, :], ob[:])

    nc.compile()
    return nc


def _get_nc():
    if "nc" not in _CACHE:
        _CACHE["nc"] = _build()
    return _CACHE["nc"]


def _run(inputs, trace=False, tmpdir=None):
    from concourse.bass_utils import run_bass_kernel_spmd

    nc = _get_nc()
    x = np.ascontiguousarray(np.asarray(inputs["x"], dtype=np.float32))
    w = np.ascontiguousarray(np.asarray(inputs["w"], dtype=np.float32))
    a = np.ascontiguousarray(np.asarray(inputs["a"], dtype=np.float32))
    core_ids = list(range(B))
    in_maps = [{"x": x[b], "w": w, "a": a} for b in core_ids]
    res = run_bass_kernel_spmd(nc, in_maps, core_ids, trace=trace, tmpdir=tmpdir)
    out = np.stack([res.results[b]["o"] for b in core_ids], axis=0)
    return out, res


def kernel(**inputs) -> np.ndarray:
    out, _ = _run(inputs, trace=False)
    return out


# revision 4
# speedup vs baseline: 1.7391x; 1.7391x over previous
"""GAT attention layer (B=8, N=2048, C=512) on 8 TRN2 NeuronCores.

Data-parallel over B: core b handles graph b.
Per-core math (x: [N,C], w: [C,C], a: [2C,1]):
    wa_t = w @ a_t                      (t=0,1)       [C]
    s_t  = x @ wa_t                                   [N]
    p_ji = exp(leaky_relu(s1_i + s2_j))
    r_i  = sum_j p_ji;  out = (p^T @ x) / r

Softmax rows are invariant to any per-row scale, so with
lambda_i = exp(-0.2*s1_i) we compute
    p'_ji = (G_i * F1_j) max F1a_j
    G = exp(0.8*s1), F1 = exp(s2), F1a = exp(0.2*s2)
which is ONE DVE tensor_scalar op per score block (two per-partition
scalars, G broadcast along the free dim) -- no per-element exp at all.

Column->row redistribution (for wa and G) uses DVE 32x32 block
transposes + row-contiguous DRAM hop + partition-broadcast DMA; naive
element-granular scatter DMAs cost ~7us each (descriptor-bound).

The score stream is split into i-halves: the first half only needs
s1[0:1024] (8 row-dots), so PE's PSUM-group-0 matmuls start ~5us
earlier; the second half's dots/TS ops hide inside the stream.

Engine roles:
  PE   : 256 PV MMs (215ns) + 256 r-MMs (25ns)  <- pacer
  DVE  : row-dots (STT+accum), block transposes, p' tensor_scalar, recip
  ACT  : x->bf16 casts, G/F exps, output normalize
  GPS  : small broadcast DMAs only (its compute poisons DVE ~3x)
"""

import sys

import numpy as np

if "/opt/trn_rl_repo" not in sys.path:
    sys.path.insert(0, "/opt/trn_rl_repo")

B, N, C = 8, 2048, 512
P = 128
NJ = N // P  # 16 source-node blocks
NH = N // 2  # 1024: i-half width
ALPHA = 0.2  # leaky_relu slope
# PSUM: 8 banks of [128, 512] fp32. 7 output accumulators + 1 r bank.
GROUPS = [(0, 7), (7, 14), (14, 16)]

_CACHE = {}


def _build():
    from contextlib import ExitStack

    import concourse.bacc as bacc
    import concourse.bass as bass
    import concourse.tile as tile
    from concourse import mybir

    fp32 = mybir.dt.float32
    bf16 = mybir.dt.bfloat16
    AF = mybir.ActivationFunctionType
    OP = mybir.AluOpType

    nc = bacc.Bacc("TRN2", target_bir_lowering=False)
    x_d = nc.dram_tensor("x", [N, C], fp32, kind="ExternalInput")
    w_d = nc.dram_tensor("w", [C, C], fp32, kind="ExternalInput")
    a_d = nc.dram_tensor("a", [2 * C, 1], fp32, kind="ExternalInput")
    o_d = nc.dram_tensor("o", [N, C], fp32, kind="ExternalOutput")

    with ExitStack() as ctx:
        tc = ctx.enter_context(tile.TileContext(nc))
        const = ctx.enter_context(tc.tile_pool(name="const", bufs=1))
        wpool = ctx.enter_context(tc.tile_pool(name="w", bufs=4))
        xpool = ctx.enter_context(tc.tile_pool(name="xin", bufs=NJ))
        xbfp = ctx.enter_context(tc.tile_pool(name="xbf", bufs=NJ))
        ppool = ctx.enter_context(tc.tile_pool(name="p", bufs=2 * NJ))
        scr = ctx.enter_context(tc.tile_pool(name="scr", bufs=3))
        osb = ctx.enter_context(tc.tile_pool(name="osb", bufs=3))
        dram = ctx.enter_context(tc.tile_pool(name="dram", bufs=1, space="DRAM"))
        ps_out = ctx.enter_context(tc.tile_pool(name="ps_out", bufs=7, space="PSUM"))
        ps_r = ctx.enter_context(tc.tile_pool(name="ps_r", bufs=1, space="PSUM"))

        # --- persistent small tiles -------------------------------------
        s1col = const.tile([P, NJ], fp32)  # s1[128j+p] at [p, j]
        s2col = const.tile([P, NJ], fp32)
        F1col = const.tile([P, NJ], fp32)  # exp(s2)
        F1acol = const.tile([P, NJ], fp32)  # exp(ALPHA*s2)
        Gpad = const.tile([P, 32], bf16)  # exp(0.8*s1) cols (j at [:,j])
        GT = const.tile([32, P], bf16)  # half-1 transpose: G[j*128+p] at [j,p]
        GT2 = const.tile([32, P], bf16)  # half-2 transpose
        Gb = const.tile([P, N], bf16)  # G broadcast rows
        wa12 = const.tile([P, 32], fp32)  # wa_t[128q+p] at [p, t*4+q]
        waT = const.tile([32, P], fp32)  # transposed: row t*4+q holds wa chunk
        abc = const.tile([P, 2, C], fp32)  # a rows broadcast to 128 parts
        wab = const.tile([P, 2, C], fp32)  # wa rows broadcast to 128 parts
        ones_bf = const.tile([P, 1], bf16)
        ones_f32 = const.tile([P, P], fp32)
        warm_rhs = const.tile([P, C], bf16)
        rinv = const.tile([P, NJ], fp32)
        dummy = const.tile([P, 1], fp32)
        dummy2 = const.tile([P, 1], fp32)

        scratch_wa = dram.tile([2 * C], fp32)
        scratch_G = dram.tile([N], bf16)

        nc.vector.memset(ones_bf[:], 1.0)
        nc.vector.memset(ones_f32[:], 1.0)
        nc.vector.memset(warm_rhs[:], 0.0)

        # Preload ACT exp table (~2.7us) during the DMA head phase.
        nc.scalar.activation(dummy[:], ones_bf[:], AF.Exp)

        # PE warm-up: back-to-back MMs trip the HAM activity window so the
        # array reaches 8/8 clock before real matmuls arrive.
        warm_ps = ps_r.tile([P, C], fp32, tag="rps", name="warm_ps")
        for _ in range(16):
            nc.tensor.matmul(
                warm_ps[0:1, :],
                lhsT=ones_bf[:],
                rhs=warm_rhs[:],
                start=True,
                stop=True,
                skip_group_check=True,
            )

        # --- a -> abc (partition-broadcast DMA straight from DRAM) ------
        a_rows = a_d[:, 0].rearrange("(t c) -> t c", t=2)  # [2, C]
        nc.gpsimd.dma_start(
            out=abc[:],
            in_=bass.AP(
                tensor=a_rows.tensor,
                offset=a_rows.offset,
                ap=[[0, P]] + list(a_rows.ap),
            ),
        )
        # --- w tiles (gpsimd ring -- sync ring is saturated by x) --------
        wt = []
        for q in range(4):
            t = wpool.tile([P, C], fp32, tag="w")
            nc.gpsimd.dma_start(t[:], w_d[q * P : (q + 1) * P, :])
            wt.append(t)

        # --- wa dots on DVE; col->row via block transpose + DRAM hop -----
        for q in range(4):
            for t in range(2):
                s = scr.tile([P, C], fp32, tag="ttr")
                nc.vector.scalar_tensor_tensor(
                    out=s[:],
                    in0=wt[q][:],
                    scalar=0.0,
                    in1=abc[:, t, :],
                    op0=OP.add,
                    op1=OP.mult,
                    accum_out=wa12[:, t * 4 + q : t * 4 + q + 1],
                )
        for b in range(4):
            nc.vector.transpose(waT[0:32, b * 32 : (b + 1) * 32],
                                wa12[b * 32 : (b + 1) * 32, 0:32])
        nc.vector.reciprocal(dummy2[:], dummy[:])  # preload DVE recip table
        nc.gpsimd.dma_start(
            out=scratch_wa[:].rearrange("(r p) -> r p", p=P),
            in_=waT[0:8, :],
        )
        wa_rows = scratch_wa[:].rearrange("(t c) -> t c", t=2)
        nc.gpsimd.dma_start(
            out=wab[:],
            in_=bass.AP(
                tensor=wa_rows.tensor,
                offset=wa_rows.offset,
                ap=[[0, P]] + list(wa_rows.ap),
            ),
        )

        # --- load x; s1 half-1 dots on DVE; casts on ACT; PE keep-alives -
        xin, xbf, sscr = [], [], []

        def emit_s1(j):
            s = scr.tile([P, C], fp32, tag="ttr", name=f"s1scr_{j}")
            nc.vector.scalar_tensor_tensor(
                out=s[:],
                in0=xin[j][:],
                scalar=0.0,
                in1=wab[:, 0, :],
                op0=OP.add,
                op1=OP.mult,
                accum_out=s1col[:, j : j + 1],
            )
            return s

        for j in range(NJ):
            t = xpool.tile([P, C], fp32, tag="xin")
            nc.sync.dma_start(t[:], x_d[j * P : (j + 1) * P, :])
            xin.append(t)
            nc.tensor.matmul(
                warm_ps[:, :],
                lhsT=ones_f32[:],
                rhs=t[:],
                start=True,
                stop=True,
                skip_group_check=True,
            )
            xb = xbfp.tile([P, C], bf16, tag="xbf")
            nc.scalar.activation(xb[:], t[:], AF.Copy)
            xbf.append(xb)
            if j < 8:
                sscr.append(emit_s1(j))

        # half-1 G: exp(0.8*s1[0:1024]) -> transpose -> DRAM -> broadcast
        nc.scalar.activation(Gpad[:, 0:8], s1col[:, 0:8], AF.Exp, scale=1.0 - ALPHA)
        for b in range(4):
            nc.vector.transpose(GT[0:32, b * 32 : (b + 1) * 32],
                                Gpad[b * 32 : (b + 1) * 32, 0:32])
        nc.gpsimd.dma_start(
            out=scratch_G[0:NH].rearrange("(j p) -> j p", p=P),
            in_=GT[0:8, :],
        )
        g1 = scratch_G[0:NH]
        nc.gpsimd.dma_start(
            out=Gb[:, 0:NH],
            in_=bass.AP(tensor=g1.tensor, offset=g1.offset, ap=[[0, P]] + list(g1.ap)),
        )
        # PE keep-alives through the dot phase (ride on s1 scratches)
        for j in (1, 3, 5, 7):
            nc.tensor.matmul(
                warm_ps[:, :],
                lhsT=ones_f32[:],
                rhs=sscr[j][:],
                start=True,
                stop=True,
                skip_group_check=True,
            )
        nc.tensor.matmul(  # keep-alive blip gated on the Gb broadcast
            warm_ps[0:1, :],
            lhsT=ones_bf[:],
            rhs=Gb[:, 0:C],
            start=True,
            stop=True,
            skip_group_check=True,
        )

        # s2 row-dots + F exps, interleaved into the stream with lookahead
        def emit_s2(j):
            s = scr.tile([P, C], fp32, tag="ttr", name=f"s2scr_{j}")
            nc.vector.scalar_tensor_tensor(
                out=s[:],
                in0=xin[j][:],
                scalar=0.0,
                in1=wab[:, 1, :],
                op0=OP.add,
                op1=OP.mult,
                accum_out=s2col[:, j : j + 1],
            )
            nc.scalar.activation(F1col[:, j : j + 1], s2col[:, j : j + 1], AF.Exp)
            nc.scalar.activation(
                F1acol[:, j : j + 1], s2col[:, j : j + 1], AF.Exp, scale=ALPHA
            )

        emit_s2(0)
        emit_s2(1)

        # --- half-1 score stream: p'[j,i] = (G_i * F1_j) max F1a_j -------
        ptA, ptB = [], []
        for j in range(NJ):
            p = ppool.tile([P, NH], bf16, tag="p", name=f"pA_{j}")
            nc.vector.tensor_scalar(
                out=p[:],
                in0=Gb[:, 0:NH],
                scalar1=F1col[:, j : j + 1],
                scalar2=F1acol[:, j : j + 1],
                op0=OP.mult,
                op1=OP.max,
            )
            ptA.append(p)
            if j < 8:  # half-2 s1 dots hide inside the stream
                emit_s1(8 + j)
            if j == 9:  # half-2 G chain (s1 complete by stream slot 9)
                nc.scalar.activation(
                    Gpad[:, 8:16], s1col[:, 8:16], AF.Exp, scale=1.0 - ALPHA
                )
                for b in range(4):
                    nc.vector.transpose(GT2[0:32, b * 32 : (b + 1) * 32],
                                        Gpad[b * 32 : (b + 1) * 32, 0:32])
                g2 = scratch_G[NH:N]
                nc.gpsimd.dma_start(
                    out=scratch_G[NH:N].rearrange("(j p) -> j p", p=P),
                    in_=GT2[8:16, :],
                )
                nc.gpsimd.dma_start(
                    out=Gb[:, NH:N],
                    in_=bass.AP(
                        tensor=g2.tensor, offset=g2.offset,
                        ap=[[0, P]] + list(g2.ap),
                    ),
                )
            if j + 2 < NJ:
                emit_s2(j + 2)

        # --- half-2 score tiles (feed PSUM groups 1-2) -------------------
        for j in range(NJ):
            p = ppool.tile([P, NH], bf16, tag="p", name=f"pB_{j}")
            nc.vector.tensor_scalar(
                out=p[:],
                in0=Gb[:, NH:N],
                scalar1=F1col[:, j : j + 1],
                scalar2=F1acol[:, j : j + 1],
                op0=OP.mult,
                op1=OP.max,
            )
            ptB.append(p)

        def lhs_chunk(j, k):
            if k < 8:
                return ptA[j][:, k * P : (k + 1) * P]
            return ptB[j][:, (k - 8) * P : (k - 7) * P]

        # --- PV + r + normalize, in PSUM-sized chunk groups --------------
        for g0, g1_ in GROUPS:
            nk = g1_ - g0
            outps = [
                ps_out.tile([P, C], fp32, tag="ops", name=f"ops_{g0}_{ki}")
                for ki in range(nk)
            ]
            rps = ps_r.tile([P, C], fp32, tag="rps")
            for j in range(NJ):
                first, last = j == 0, j == NJ - 1
                for ki, k in enumerate(range(g0, g1_)):
                    lhs = lhs_chunk(j, k)
                    nc.tensor.matmul(
                        outps[ki][:], lhsT=lhs, rhs=xbf[j][:], start=first, stop=last
                    )
                    # start=True clears the WHOLE bank's has_written bits, so
                    # only the very first matmul into this bank may set it;
                    # later first-touches per element overwrite (bit clear)
                    # and the rest accumulate.
                    nc.tensor.matmul(
                        rps[:, ki : ki + 1],
                        lhsT=lhs,
                        rhs=ones_bf[:],
                        start=first and ki == 0,
                        stop=last,
                        skip_group_check=True,
                    )
            nc.vector.reciprocal(rinv[:, g0:g1_], rps[:, :nk])
            for ki, k in enumerate(range(g0, g1_)):
                ob = osb.tile([P, C], fp32, tag="ob")
                nc.scalar.activation(
                    ob[:], outps[ki][:], AF.Copy, bias=0.0, scale=rinv[:, k : k + 1]
                )
                nc.sync.dma_start(o_d[k * P : (k + 1) * P, :], ob[:])

    nc.compile()
    return nc


def _get_nc():
    if "nc" not in _CACHE:
        _CACHE["nc"] = _build()
    return _CACHE["nc"]


def _run(inputs, trace=False, tmpdir=None):
    from concourse.bass_utils import run_bass_kernel_spmd

    nc = _get_nc()
    x = np.ascontiguousarray(np.asarray(inputs["x"], dtype=np.float32))
    w = np.ascontiguousarray(np.asarray(inputs["w"], dtype=np.float32))
    a = np.ascontiguousarray(np.asarray(inputs["a"], dtype=np.float32))
    core_ids = list(range(B))
    in_maps = [{"x": x[b], "w": w, "a": a} for b in core_ids]
    res = run_bass_kernel_spmd(nc, in_maps, core_ids, trace=trace, tmpdir=tmpdir)
    out = np.stack([res.results[b]["o"] for b in core_ids], axis=0)
    return out, res


def kernel(**inputs) -> np.ndarray:
    out, _ = _run(inputs, trace=False)
    return out
